# revision 1
# baseline (speedup 1.0000x reference)
"""Stride-2 bilinear upsampling (block-diagonal conv_transpose2d) on 8 NeuronCores.

The reference op is F.conv_transpose2d(x, w, stride=2) where w is
block-diagonal: w[c, c] = filt (4x4 separable bilinear tap), zero
off-diagonal.  So the op is a per-channel depthwise separable upsample:

    out[2m]   = k0*x[m] + k2*x[m-1]
    out[2m+1] = k1*x[m] + k3*x[m-1]        (along H and W independently)

with k = [0.25, 0.75, 0.75, 0.25] (so k0 == k3, k1 == k2).

Sharding: channel-parallel, 32 channels x 4 batch = 128 independent
128x128 images per core, one image per SBUF partition.  Each core runs a
separable two-pass upsample (W-pass then H-pass) over 8 horizontal
strips, with scaled copies on ScalarE and the two-term blends as
scalar_tensor_tensor on VectorE.  DMA-bound at ~43MB HBM traffic/core.
"""

import numpy as np

N, C, H, W = 4, 256, 128, 128
OH, OW = 258, 258
NCORES = 8
CPC = C // NCORES          # 32 channels per core
NIMG = N * CPC             # 128 images per core (one per SBUF partition)
NSTRIPS = 8
HS = 16                    # output-row-pairs (m values) per strip; last strip 17

_CACHE = {}


def _legalize_waits(nc, mybir):
    """Split multi-wait sync_info into standalone single-wait EventSemaphore
    instructions.  This walrus build encodes at most one sync-wait command per
    instruction ("Too many sync wait commands" in setupSyncWait otherwise);
    engines are in-order, so hoisting extra waits into preceding same-engine
    instructions is semantics-preserving."""
    n = 0
    for func in nc.m.functions:
        for block in func.blocks:
            out = []
            for inst in block.instructions:
                si = inst.sync_info
                if si is not None and si.on_wait is not None and len(si.on_wait) > 1:
                    waits = list(si.on_wait)
                    for k, w in enumerate(waits[:-1]):
                        out.append(mybir.InstEventSemaphore(
                            name=f"{inst.name}-hw{k}",
                            opcode="EventSemaphore",
                            engine=inst.engine,
                            ins=[], outs=[],
                            sync_info=mybir.SyncInfo(on_wait=[w], on_update=[]),
                        ))
                        n += 1
                    inst.sync_info = mybir.SyncInfo(
                        on_wait=[waits[-1]], on_update=list(si.on_update))
                out.append(inst)
            block.instructions = out
    return n


def _build_bass(f0, f2, hs=HS, bufs=2, bufs_z=None, bufs_x=None, bufs_q=None,
                bufs_y=None, bufs_q2=None, in_ring="sync", hoist_in=False,
                split_z=1, repeat=1, dve_frac=1.0, out_frac=1.0, act_frac=1.0,
                big_x=False, in_chunks=4, out_ring="sync"):
    """Build the SPMD Bass program (per-core view: x[128,128,128] -> out[128,258,258]).

    f0 = tap on x[m] for even outputs (== tap on x[m-1] for odd outputs)
    f2 = tap on x[m-1] for even outputs (== tap on x[m] for odd outputs)
    """
    import concourse.bass as bass
    import concourse.mybir as mybir
    from concourse.tile import TileContext

    f32 = mybir.dt.float32
    Copy = mybir.ActivationFunctionType.Copy
    mult, add = mybir.AluOpType.mult, mybir.AluOpType.add
    if bufs_z is None:
        bufs_z = bufs
    bufs_x = bufs_x or bufs
    bufs_q = bufs_q or bufs
    bufs_y = bufs_y or bufs
    bufs_q2 = bufs_q2 or bufs
    nstrips = H // hs

    nc = bass.Bass()
    x = nc.dram_tensor("x", [NIMG, H, W], f32, kind="ExternalInput")
    out = nc.dram_tensor("out", [NIMG, OH, OW], f32, kind="ExternalOutput")
    # benchmark mode: repeat the whole computation; non-final reps write to
    # internal DRAM scratch so reps don't serialize on output WAW deps
    scratch = [nc.dram_tensor(f"scr{i}", [NIMG, OH, OW], f32, kind="Internal")
               for i in range(min(2, repeat - 1))]

    with TileContext(nc) as tc:
        with tc.tile_pool(name="p", bufs=bufs) as pool:
            in_eng = {"sync": nc.sync, "scalar": nc.scalar,
                      "gpsimd": nc.gpsimd, "tensor": nc.tensor}[in_ring]

            def strip_geom(s):
                m0 = s * hs
                n_m = hs if s < nstrips - 1 else hs + 1   # output row-pairs
                return m0, n_m, n_m + 1                   # rows incl. halo

            def load_x(s):
                m0, n_m, rows = strip_geom(s)
                xt = pool.tile([NIMG, rows, W], f32, tag="xt", bufs=bufs_x)
                if s == 0:
                    nc.vector.memset(xt[:, 0:1, :], 0.0)          # X[-1] = 0
                    in_eng.dma_start(out=xt[:, 1:rows, :], in_=x[:, 0:n_m, :])
                elif s == nstrips - 1:
                    nc.vector.memset(xt[:, rows - 1:rows, :], 0.0)  # X[128] = 0
                    in_eng.dma_start(out=xt[:, 0:rows - 1, :],
                                     in_=x[:, m0 - 1:m0 + n_m - 1, :])
                else:
                    in_eng.dma_start(out=xt[:, :, :],
                                     in_=x[:, m0 - 1:m0 + n_m, :])
                return xt

            xbig = None
            if big_x:
                # one persistent input tile [img, 130, W]: row i = X[i-1];
                # rows 0 and H+1 are zero ghosts, loads target rows 1..H
                xbig = pool.tile([NIMG, H + 2, W], f32, tag="xbig", bufs=1)
                nc.vector.memset(xbig[:, 0:1, :], 0.0)
                nc.vector.memset(xbig[:, H + 1:H + 2, :], 0.0)

            for rep in range(repeat):
                tgt = out if rep == repeat - 1 else scratch[rep % 2]
                xts = {}
                if big_x:
                    for c in range(in_chunks):
                        r0 = H * c // in_chunks
                        r1 = H * (c + 1) // in_chunks
                        in_eng.dma_start(out=xbig[:, 1 + r0:1 + r1, :],
                                         in_=x[:, r0:r1, :])
                elif hoist_in:
                    for s in range(nstrips):
                        xts[s] = load_x(s)

                for s in range(nstrips):
                    m0, n_m, rows = strip_geom(s)
                    if big_x:
                        xt = xbig[:, m0:m0 + rows, :]
                    else:
                        xt = xts[s] if hoist_in else load_x(s)

                    # ---- W-pass: Y[r, 2m]   = f0*X[r, m] + f2*X[r, m-1]
                    #              Y[r, 2m+1] = f2*X[r, m] + f0*X[r, m-1]
                    qt = pool.tile([NIMG, rows, W], f32, tag="qt", bufs=bufs_q)    # f2 * X
                    ra = max(1, int(round(rows * act_frac)))
                    nc.scalar.activation(qt[:, :ra, :], xt[:, :ra, :], Copy, scale=f2)

                    yt = pool.tile([NIMG, rows, OW], f32, tag="yt", bufs=bufs_y)
                    rv = max(1, int(round(rows * dve_frac)))
                    # even body m=1..127
                    nc.vector.scalar_tensor_tensor(
                        out=yt[:, :rv, 2:2 * W:2], in0=xt[:, :rv, 1:W], scalar=f0,
                        in1=qt[:, :rv, 0:W - 1], op0=mult, op1=add)
                    # odd body m=1..127
                    nc.vector.scalar_tensor_tensor(
                        out=yt[:, :rv, 3:2 * W + 1:2], in0=xt[:, :rv, 0:W - 1], scalar=f0,
                        in1=qt[:, :rv, 1:W], op0=mult, op1=add)
                    # edges: m=0 and m=128
                    nc.scalar.activation(yt[:, :, 0:1], xt[:, :, 0:1], Copy, scale=f0)
                    nc.scalar.copy(yt[:, :, 1:2], qt[:, :, 0:1])
                    nc.scalar.copy(yt[:, :, 2 * W:2 * W + 1], qt[:, :, W - 1:W])
                    nc.scalar.activation(yt[:, :, 2 * W + 1:2 * W + 2],
                                         xt[:, :, W - 1:W], Copy, scale=f0)

                    # ---- H-pass: Z[2m]   = f0*Y[m] + f2*Y[m-1]
                    #              Z[2m+1] = f2*Y[m] + f0*Y[m-1]
                    q2t = pool.tile([NIMG, rows, OW], f32, tag="q2t", bufs=bufs_q2)  # f2 * Y
                    nc.scalar.activation(q2t[:, :ra, :], yt[:, :ra, :], Copy, scale=f2)

                    # split Z into chunks of m-values for finer DMA pipelining
                    nch = split_z
                    bounds = [n_m * c // nch for c in range(nch + 1)]
                    for c in range(nch):
                        j0, j1 = bounds[c], bounds[c + 1]
                        nj = j1 - j0
                        if nj == 0:
                            continue
                        zt = pool.tile([NIMG, 2 * nj, OW], f32, tag="zt",
                                       bufs=bufs_z)
                        njv = max(1, int(round(nj * dve_frac)))
                        nc.vector.scalar_tensor_tensor(
                            out=zt[:, 0:2 * njv:2, :], in0=yt[:, 1 + j0:1 + j0 + njv, :],
                            scalar=f0, in1=q2t[:, j0:j0 + njv, :], op0=mult, op1=add)
                        nc.vector.scalar_tensor_tensor(
                            out=zt[:, 1:2 * njv:2, :], in0=yt[:, j0:j0 + njv, :],
                            scalar=f0, in1=q2t[:, 1 + j0:1 + j0 + njv, :], op0=mult,
                            op1=add)
                        njo = max(1, int(round(nj * out_frac)))
                        oeng = (nc.scalar if out_ring == "alt" and s % 2
                                else nc.sync)
                        oeng.dma_start(
                            out=tgt[:, 2 * (m0 + j0):2 * (m0 + j0 + njo), :],
                            in_=zt[:, 0:2 * njo, :])

    _legalize_waits(nc, mybir)
    return nc


def _taps_from_w(w):
    """Recover the separable 4-tap filter f (filt = outer(f, f)) from w[0, 0]."""
    filt = np.asarray(w, dtype=np.float32)[0, 0]
    j = int(np.argmax(np.abs(np.diag(filt))))
    f = filt[:, j] / np.float32(np.sqrt(filt[j, j]))
    # sanity: separable and symmetric (k0==k3, k1==k2) as the kernel assumes
    assert np.allclose(np.outer(f, f), filt, atol=1e-5), "filter not separable"
    assert abs(f[0] - f[3]) < 1e-6 and abs(f[1] - f[2]) < 1e-6, "filter not symmetric"
    return float(f[0]), float(f[2])


BEST_CFG = dict(hs=8, bufs=2, bufs_x=8, in_ring='scalar', hoist_in=True)


def _get_nc(f0, f2, **cfg):
    cfg = {**BEST_CFG, **cfg}
    key = (round(f0, 8), round(f2, 8), tuple(sorted(cfg.items())))
    if key not in _CACHE:
        _CACHE[key] = _build_bass(f0, f2, **cfg)
    return _CACHE[key]


def run_sharded(x, w, **run_kwargs):
    """Shard, run on 8 cores, gather.  Extra kwargs go to run_bass_kernel_spmd."""
    from concourse.bass_utils import run_bass_kernel_spmd

    x = np.ascontiguousarray(np.asarray(x, dtype=np.float32))
    f0, f2 = _taps_from_w(w)
    nc = _get_nc(f0, f2)

    in_maps = []
    for k in range(NCORES):
        xk = np.ascontiguousarray(
            x[:, k * CPC:(k + 1) * CPC].reshape(NIMG, H, W))
        in_maps.append({"x": xk})

    res = run_bass_kernel_spmd(nc, in_maps, core_ids=list(range(NCORES)),
                               **run_kwargs)

    full = np.empty((N, C, OH, OW), dtype=np.float32)
    for k in range(NCORES):
        full[:, k * CPC:(k + 1) * CPC] = res.results[k]["out"].reshape(
            N, CPC, OH, OW)
    return full, res


def kernel(x, w):
    full, _ = run_sharded(x, w)
    return full

